# revision 18
# baseline (speedup 1.0000x reference)
"""LIF (leaky integrate-and-fire) forward kernel for Trainium2, 8 NeuronCores.

Recurrence (per element of [B, N], serial over T):
    v_t = DECAY * w_{t-1} + x_t          (REST = 0, w = post-reset membrane)
    s_t = (v_t > THRESHOLD)
    w_t = v_t * (v_t <= THRESHOLD)

Engine plan (per core, per step tile of [128 partitions, 2048]):
  - DVE: the two fused scalar_tensor_tensor ops of the recurrence. This is
    the serial critical path (~2.3us/op); DVE is the only engine that runs
    fused two-tensor fp32 elementwise at full rate (Pool is ~4x slower and
    steals SBUF ports, ACT's ISA only allows activations, PE fp32 matmul
    offload adds a 3-engine cycle that schedules worse than DVE alone).
  - ScalarE: spike as Sign(v - THR) in fp8 {-1, 0, 1} (off-chain).
  - PE: packs 8 consecutive steps' signs into one byte-plane via
    accumulating matmuls with stationary weights 2^k * I (fp8) into PSUM;
    ScalarE converts (psum + 255)/2 -> uint8. Output traffic is 8x smaller
    than fp8 spikes, keeping DMA far below the chain.
  - Host decodes bit k of byte-plane g as the spike at t = 8*g + k.

All recurrence arithmetic is fp32 and bitwise-faithful to the reference
ordering. (A byte can only be corrupted if some v_t == THR exactly, which
Sign maps to 0; measure-zero in practice and far inside the 2e-2 budget.)

Head/tail: the t=0 load is split across both HWDGE queues (sync + scalar)
so compute starts ~2 transfers earlier; the last step runs in column halves
so sign -> pack -> convert -> store pipelines out behind the final STT.

Sharding: batch dim (128) split 16 rows/core across 8 cores; per-core,
per-step slab is a contiguous 1 MiB block viewed as [128 partitions, 2048].
"""

import numpy as np

import concourse.bacc as bacc
import concourse.mybir as mybir
from concourse.tile import TileContext
from concourse.bass_utils import run_bass_kernel_spmd

T, B, N = 32, 128, 16384
N_CORES = 8
B_SH = B // N_CORES          # 16 batch rows per core
S = B_SH * N                 # 262144 elements per core per time step
P = 128                      # SBUF partitions
F = S // P                   # 2048 free-dim elements
G = T // 8                   # packed byte groups
DECAY = 0.2
THR = 0.3

TRACE = False                # set True (e.g. from test.py) to capture a profile

_BUILT = {}


def _build_nc():
    nc = bacc.Bacc("TRN2", debug=False, num_devices=N_CORES)
    x = nc.dram_tensor("x", [T, S], mybir.dt.float32, kind="ExternalInput").ap()
    y = nc.dram_tensor("y", [G, S], mybir.dt.uint8, kind="ExternalOutput").ap()
    xr = x.rearrange("t (p f) -> t p f", p=P)
    yr = y.rearrange("g (p f) -> g p f", p=P)

    f32 = mybir.dt.float32
    fp8 = mybir.dt.float8e4
    Alu = mybir.AluOpType
    Act = mybir.ActivationFunctionType

    H = F // 2
    with TileContext(nc) as tc:
        with (
            tc.tile_pool(name="state", bufs=1) as state_pool,
            tc.tile_pool(name="xin", bufs=10) as xin_pool,
            tc.tile_pool(name="vtmp", bufs=3) as v_pool,
            tc.tile_pool(name="st", bufs=3) as st_pool,
            tc.tile_pool(name="ob", bufs=2) as ob_pool,
            tc.tile_pool(name="pk", bufs=2, space="PSUM") as psum_pool,
        ):
            negthr = nc.alloc_sbuf_tensor("const_negthr", [P, 1], f32).ap()
            nc.vector.memset(negthr, -THR)

            # Pack weights: wk[k] = 2^k * I in fp8. One Pool iota builds
            # d[p,f] = p - f; tiny DVE tensor_scalars turn it into the eight
            # scaled identities ((d==0) * 2^k). Persistent allocations.
            wtmp = nc.alloc_sbuf_tensor("wk_iota", [P, 128], f32).ap()
            nc.gpsimd.iota(
                wtmp, pattern=[[-1, 128]], base=0, channel_multiplier=1,
                allow_small_or_imprecise_dtypes=True,
            )
            wks = []
            for k in range(8):
                wk = nc.alloc_sbuf_tensor(f"wk_{k}", [P, 128], fp8).ap()
                nc.vector.tensor_scalar(
                    out=wk, in0=wtmp, scalar1=0.0, scalar2=float(1 << k),
                    op0=Alu.is_equal, op1=Alu.mult,
                )
                wks.append(wk)

            w = state_pool.tile([P, F], f32)

            ps = None
            for t in range(T):
                g, k = divmod(t, 8)
                xt = xin_pool.tile([P, F], f32)
                if t == 0:
                    # quarter the first load so compute starts on 256 KiB
                    for j in range(0, F, 512):
                        nc.sync.dma_start(out=xt[:, j:j + 512], in_=xr[t][:, j:j + 512])
                elif t == 1:
                    # second load rides the idle ACT HWDGE queue so it isn't
                    # serialized behind the t=0 quarters
                    nc.scalar.dma_start(out=xt[:], in_=xr[t])
                else:
                    nc.sync.dma_start(out=xt[:], in_=xr[t])

                st = st_pool.tile([P, F], fp8)
                if t == 0:
                    # w_{-1}=0 so v_0 = x_0: read x directly, per quarter
                    for j in range(0, F, 512):
                        nc.vector.scalar_tensor_tensor(
                            out=w[:, j:j + 512], in0=xt[:, j:j + 512], scalar=THR,
                            in1=xt[:, j:j + 512], op0=Alu.is_le, op1=Alu.mult,
                        )
                        nc.scalar.activation(
                            st[:, j:j + 512], xt[:, j:j + 512], Act.Sign, bias=negthr
                        )
                elif t < T - 1:
                    v = v_pool.tile([P, F], f32)
                    # v = w*DECAY + x
                    nc.vector.scalar_tensor_tensor(
                        out=v[:], in0=w[:], scalar=DECAY, in1=xt[:],
                        op0=Alu.mult, op1=Alu.add,
                    )
                    # w = (v<=THR)*v
                    nc.vector.scalar_tensor_tensor(
                        out=w[:], in0=v[:], scalar=THR, in1=v[:],
                        op0=Alu.is_le, op1=Alu.mult,
                    )
                    nc.scalar.activation(st[:], v[:], Act.Sign, bias=negthr)
                else:
                    # last step: w is dead; quarter so the tail pipelines out
                    v = v_pool.tile([P, F], f32)
                    for j in range(0, F, 512):
                        nc.vector.scalar_tensor_tensor(
                            out=v[:, j:j + 512], in0=w[:, j:j + 512], scalar=DECAY,
                            in1=xt[:, j:j + 512], op0=Alu.mult, op1=Alu.add,
                        )
                        nc.scalar.activation(
                            st[:, j:j + 512], v[:, j:j + 512], Act.Sign, bias=negthr
                        )

                # pack: psum bank j accumulates 2^k * st (identity matmul)
                if k == 0:
                    ps = psum_pool.tile([P, F], f32)
                for j in range(0, F, 512):
                    nc.tensor.matmul(
                        out=ps[:, j:j + 512], lhsT=wks[k][:], rhs=st[:, j:j + 512],
                        start=(k == 0), stop=(k == 7),
                    )
                if k == 7:
                    ob = ob_pool.tile([P, F], mybir.dt.uint8)
                    # (sum_k 2^k sign_k + 255) / 2 -> byte of spike bits
                    if t == T - 1:
                        # stream out per psum bank, alternating HWDGE queues
                        # (the sync queue has no input loads left by now)
                        for i, j in enumerate(range(0, F, 512)):
                            nc.scalar.activation(
                                ob[:, j:j + 512], ps[:, j:j + 512], Act.Copy,
                                bias=127.5, scale=0.5,
                            )
                            q = nc.sync if i % 2 == 0 else nc.scalar
                            q.dma_start(
                                out=yr[g][:, j:j + 512], in_=ob[:, j:j + 512]
                            )
                    else:
                        nc.scalar.activation(
                            ob[:], ps[:], Act.Copy, bias=127.5, scale=0.5
                        )
                        nc.scalar.dma_start(out=yr[g], in_=ob[:])
    nc.compile()
    return nc


LAST_RESULTS = None


def kernel(tx):
    global LAST_RESULTS
    tx = np.asarray(tx)
    assert tx.shape == (T, B, N) and tx.dtype == np.float32

    if "nc" not in _BUILT:
        _BUILT["nc"] = _build_nc()
    nc = _BUILT["nc"]

    in_maps = [
        {"x": np.ascontiguousarray(tx[:, c * B_SH:(c + 1) * B_SH, :]).reshape(T, S)}
        for c in range(N_CORES)
    ]
    res = run_bass_kernel_spmd(nc, in_maps, core_ids=list(range(N_CORES)), trace=TRACE)
    LAST_RESULTS = res

    out = np.empty((T, B, N), dtype=np.float32)
    for c in range(N_CORES):
        packed = np.asarray(res.results[c]["y"]).reshape(G, B_SH, N, 1)
        bits = np.unpackbits(packed, axis=3, bitorder="little")  # [G, B_SH, N, 8]
        sp = np.moveaxis(bits, 3, 1).reshape(T, B_SH, N)
        out[:, c * B_SH:(c + 1) * B_SH, :] = sp
    return out


# revision 19
# speedup vs baseline: 1.2353x; 1.2353x over previous
"""LIF (leaky integrate-and-fire) forward kernel for Trainium2, 8 NeuronCores.

Recurrence (per element of [B, N], serial over T):
    v_t = DECAY * (v_{t-1} * (1 - s_{t-1})) + x_t      (REST = 0)
    s_t = (v_t > THRESHOLD)

Reformulated with state w_t = v_t * [v_t <= THRESHOLD] (post-reset membrane):
    v_t = (w_{t-1} * DECAY) + x_t        -> one fused scalar_tensor_tensor (DVE)
    w_t = (v_t is_le THR) * v_t          -> one fused scalar_tensor_tensor (DVE)
    out = Sign(v_t - THR)                -> ScalarE activation, fp8 {-1,0,1}
Host decodes spikes as (out > 0). All arithmetic is fp32 and bitwise-faithful
to the reference ordering.

The kernel is bound by the serial DVE chain (~2.29us per fused op; DVE is the
only engine running two-tensor fp32 elementwise at full rate -- Pool is ~4x
slower and contends for SBUF, ACT's ISA only allows activations, and PE
offload adds cross-engine cycles that schedule worse). Optimizations vs the
plain loop: the first load/compute is split so the chain starts early, the
last step drops the dead w-update and is quartered so the tail
(sign -> store) pipelines out behind the final v-op.

Sharding: batch dim (128) split 16 rows/core across 8 cores; per-core,
per-step slab is a contiguous 1 MiB block viewed as [128 partitions, 2048].
"""

import numpy as np

import concourse.bacc as bacc
import concourse.mybir as mybir
from concourse.tile import TileContext
from concourse.bass_utils import run_bass_kernel_spmd

T, B, N = 32, 128, 16384
N_CORES = 8
B_SH = B // N_CORES          # 16 batch rows per core
S = B_SH * N                 # 262144 elements per core per time step
P = 128                      # SBUF partitions
F = S // P                   # 2048 free-dim elements
DECAY = 0.2
THR = 0.3

TRACE = False                # set True (e.g. from test.py) to capture a profile

_BUILT = {}


def _build_nc():
    nc = bacc.Bacc("TRN2", debug=False, num_devices=N_CORES)
    x = nc.dram_tensor("x", [T, S], mybir.dt.float32, kind="ExternalInput").ap()
    y = nc.dram_tensor("y", [T, S], mybir.dt.float8e4, kind="ExternalOutput").ap()
    xr = x.rearrange("t (p f) -> t p f", p=P)
    yr = y.rearrange("t (p f) -> t p f", p=P)

    f32 = mybir.dt.float32
    Alu = mybir.AluOpType
    Act = mybir.ActivationFunctionType

    H = F // 2
    with TileContext(nc) as tc:
        with (
            tc.tile_pool(name="state", bufs=1) as state_pool,
            tc.tile_pool(name="xin", bufs=10) as xin_pool,
            tc.tile_pool(name="vtmp", bufs=4) as v_pool,
            tc.tile_pool(name="sout", bufs=8) as s_pool,
        ):
            negthr = nc.alloc_sbuf_tensor("const_negthr", [P, 1], f32).ap()
            nc.gpsimd.memset(negthr, -THR)
            w = state_pool.tile([P, F], f32)
            for t in range(T):
                xt = xin_pool.tile([P, F], f32)
                if t == 0:
                    # split the first load so compute can start sooner
                    nc.sync.dma_start(out=xt[:, :H], in_=xr[t][:, :H])
                    nc.sync.dma_start(out=xt[:, H:], in_=xr[t][:, H:])
                else:
                    nc.sync.dma_start(out=xt[:], in_=xr[t])

                v = v_pool.tile([P, F], f32)
                st = s_pool.tile([P, F], mybir.dt.float8e4)
                if t == 0:
                    # w_{-1}=0 so v_0 = x_0: skip STT-A, read x directly
                    for c0, c1 in ((0, H), (H, F)):
                        nc.vector.scalar_tensor_tensor(
                            out=w[:, c0:c1], in0=xt[:, c0:c1], scalar=THR,
                            in1=xt[:, c0:c1], op0=Alu.is_le, op1=Alu.mult,
                        )
                        nc.scalar.activation(
                            st[:, c0:c1], xt[:, c0:c1], Act.Sign, bias=negthr
                        )
                        nc.sync.dma_start(
                            out=yr[t][:, c0:c1], in_=st[:, c0:c1]
                        )
                elif t == T - 1:
                    # tail latency trim: w is dead here (no step follows),
                    # so only v + sign remain; quarter them so the last
                    # sign/store pipeline behind the final v-ops
                    for j in range(0, F, 512):
                        nc.vector.scalar_tensor_tensor(
                            out=v[:, j:j + 512], in0=w[:, j:j + 512], scalar=DECAY,
                            in1=xt[:, j:j + 512], op0=Alu.mult, op1=Alu.add,
                        )
                        nc.scalar.activation(
                            st[:, j:j + 512], v[:, j:j + 512], Act.Sign, bias=negthr
                        )
                        nc.sync.dma_start(
                            out=yr[t][:, j:j + 512], in_=st[:, j:j + 512]
                        )
                else:
                    # v = (w * DECAY) + x
                    nc.vector.scalar_tensor_tensor(
                        out=v[:], in0=w[:], scalar=DECAY, in1=xt[:],
                        op0=Alu.mult, op1=Alu.add,
                    )
                    # w = (v is_le THR) * v
                    nc.vector.scalar_tensor_tensor(
                        out=w[:], in0=v[:], scalar=THR, in1=v[:],
                        op0=Alu.is_le, op1=Alu.mult,
                    )
                    # spike encoding: Sign(v-THR) fp8; host decodes (>0)
                    nc.scalar.activation(st[:], v[:], Act.Sign, bias=negthr)
                    nc.sync.dma_start(out=yr[t], in_=st[:])
    nc.compile()
    return nc


LAST_RESULTS = None


def kernel(tx):
    global LAST_RESULTS
    tx = np.asarray(tx)
    assert tx.shape == (T, B, N) and tx.dtype == np.float32

    if "nc" not in _BUILT:
        _BUILT["nc"] = _build_nc()
    nc = _BUILT["nc"]

    in_maps = [
        {"x": np.ascontiguousarray(tx[:, c * B_SH:(c + 1) * B_SH, :]).reshape(T, S)}
        for c in range(N_CORES)
    ]
    res = run_bass_kernel_spmd(nc, in_maps, core_ids=list(range(N_CORES)), trace=TRACE)
    LAST_RESULTS = res

    out = np.empty((T, B, N), dtype=np.float32)
    for c in range(N_CORES):
        sgn = np.asarray(res.results[c]["y"]).reshape(T, B_SH, N)
        out[:, c * B_SH:(c + 1) * B_SH, :] = (sgn > 0).astype(np.float32)
    return out
